# revision 26
# baseline (speedup 1.0000x reference)
"""CORLoss Trainium2 kernel (v3).

Reference (per row of N=128):
    mean1 = mean(d1) + EPS ; mean2 = mean(d2) + EPS
    std1, std2 unbiased ; cov = sum((d1-mean1)*(d2-mean2))/(n-1)
    cor  = (cov / (std1*std2 + EPS)) ** 3
    tl1  = -log((cor + 1 + EPS)/2)
    tl2  = mean(|softmax(d1) - softmax(d2)|)
    a = |cor| ; loss_row = a*tl1 + (1-a)*tl2
    out  = sum(loss_row) over all B rows, shape (1,)

Strategy: data-parallel over 8 NeuronCores, 16384 rows/core, streamed as
[128 partitions, 16 blocks, 128] supertiles (one row per (partition,
block)).  Per supertile:

  DMA (SWDGE)   fp32->bf16 cast during load (halves HBM bytes); quad-
                buffered mega tiles give the Pool desc-gen 3 STs of
                lookahead
  ACT           one paired Exp op (e1,e2), one paired Square op
                (sq1,sq2), Abs(g) -- all in one table set, no reloads
  DVE           everything else, kept op-lean (HW pays ~us-level per-op
                overhead): prod = d1*d2 (bf16 2x); two split fold trees
                (e-tree feeds the critical c chain early, r-tree has
                slack): 3 bf16 halving levels + one segmented fp32
                reduce; f = c*e2 in ONE tensor_tensor against a (c,c)
                bf16 pair broadcast (innermost step-1 keeps 2x mode);
                g = e1 - f; T-tree over |g|
  epilogue      per-row cor/tl1/tl2/loss on [128,128] stat tiles;
                one [128,1] partial per core; host adds 8*128 partials.
"""

import sys

sys.path.insert(0, "/opt/trn_rl_repo")

import numpy as np

import concourse.bass as bass
import concourse.tile as tile
from concourse import mybir

B, N = 131072, 128
EPS = 1e-3
N_CORES = 8
R = B // N_CORES          # rows per core = 16384
ST_ROWS = 2048            # rows per supertile
NB = ST_ROWS // 128       # 16 row-blocks per supertile
NST = R // ST_ROWS        # 8 supertiles per core
NCOLS = R // 128          # 128 stat columns per core
F32 = mybir.dt.float32
BF16 = mybir.dt.bfloat16
Alu = mybir.AluOpType
Act = mybir.ActivationFunctionType

USE_SWDGE_LOADS = True    # fp32->bf16 cast during DMA (SWDGE)


def _tt(nc, out, a, b, op):
    nc.vector.tensor_tensor(out=out, in0=a, in1=b, op=op)


def split_waits(nc, cap=1):
    """This walrus build rejects instructions carrying more than ~1 inline
    semaphore wait; move excess waits onto fresh same-engine nops placed
    immediately before the instruction."""
    for fn in nc.m.functions:
        for bb in fn.blocks:
            snapshot = list(bb.instructions)
            out = []
            for inst in snapshot:
                si = inst.sync_info
                if si is not None and si.on_wait and len(si.on_wait) > cap:
                    waits = list(si.on_wait)
                    extra, keep = waits[:-cap], waits[-cap:]
                    while si.on_wait:
                        si.on_wait.pop()
                    for w in keep:
                        si.on_wait.append(w)
                    for w in extra:
                        bi = nc.engines[inst.engine].nop(nofuse=True, hint="wsplit")
                        nop_inst = bi.ins
                        for fb in nc.m.functions[0].blocks:
                            if fb.instructions and fb.instructions[-1] is nop_inst:
                                fb.instructions.pop()
                                break
                        nop_inst.sync_info = mybir.SyncInfo(on_wait=[w], on_update=[])
                        out.append(nop_inst)
                out.append(inst)
            bb.instructions[:] = out


def _build_program(loop_k=None, unroll=None):
    nc = bass.Bass()
    d1 = nc.dram_tensor("d1", [R, N], F32, kind="ExternalInput")
    d2 = nc.dram_tensor("d2", [R, N], F32, kind="ExternalInput")
    y = nc.dram_tensor("y", [128, 1], F32, kind="ExternalOutput")

    with tile.TileContext(nc) as tc:
        with (
            tc.tile_pool(name="mega", bufs=4) as mega_pool,
            tc.tile_pool(name="data", bufs=2) as data_pool,
            tc.tile_pool(name="fold", bufs=2) as fold_pool,
            tc.tile_pool(name="fg", bufs=2) as fg_pool,
            tc.tile_pool(name="small", bufs=2) as small_pool,
            tc.tile_pool(name="stats", bufs=1) as stats_pool,
            tc.tile_pool(name="epi", bufs=1) as epi_pool,
        ):
            # statsA rows: 0=d1 1=d2 2=sq1 3=sq2 4=e1 5=e2 6=prod
            statsA = stats_pool.tile([128, 7, NCOLS], F32, tag="statsA", name="statsA")
            ta = stats_pool.tile([128, 1, NCOLS], F32, tag="ta", name="ta")

            def tree(pool, src, nch, prefix, out_col, deep=False):
                """bf16 halving tree src [128,nch,NB,128] then one fp32
                segmented reduce into out_col [128,nch,NB]."""
                t1_ = pool.tile([128, nch, NB, 64], BF16, tag=f"{prefix}1", name=f"{prefix}1")
                _tt(nc, t1_, src[:, :, :, 0:64], src[:, :, :, 64:128], Alu.add)
                t2_ = pool.tile([128, nch, NB, 32], BF16, tag=f"{prefix}2", name=f"{prefix}2")
                _tt(nc, t2_, t1_[:, :, :, 0:32], t1_[:, :, :, 32:64], Alu.add)
                t3_ = pool.tile([128, nch, NB, 16], BF16, tag=f"{prefix}3", name=f"{prefix}3")
                _tt(nc, t3_, t2_[:, :, :, 0:16], t2_[:, :, :, 16:32], Alu.add)
                last = t3_
                if deep:
                    t4_ = pool.tile([128, nch, NB, 8], BF16, tag=f"{prefix}4", name=f"{prefix}4")
                    _tt(nc, t4_, t3_[:, :, :, 0:8], t3_[:, :, :, 8:16], Alu.add)
                    t5_ = pool.tile([128, nch, NB, 4], BF16, tag=f"{prefix}5", name=f"{prefix}5")
                    _tt(nc, t5_, t4_[:, :, :, 0:4], t4_[:, :, :, 4:8], Alu.add)
                    last = t5_
                nc.vector.reduce_sum(out=out_col, in_=last, axis=mybir.AxisListType.X)

            def one_supertile(st):
                rows = slice(st * ST_ROWS, (st + 1) * ST_ROWS)
                cols = slice(st * NB, (st + 1) * NB)
                src1 = d1[rows, :].rearrange("(p b) n -> p b n", p=128)
                src2 = d2[rows, :].rearrange("(p b) n -> p b n", p=128)

                # megaR: 0=d1 1=d2 2=sq1 3=sq2 4=prod ; megaE: 0=e1 1=e2
                megaR = mega_pool.tile([128, 5, NB, N], BF16, tag="megaR", name="megaR")
                megaE = mega_pool.tile([128, 2, NB, N], BF16, tag="megaE", name="megaE")
                t1 = megaR[:, 0]
                t2 = megaR[:, 1]
                nc.gpsimd.dma_start(out=t1, in_=src1)
                nc.gpsimd.dma_start(out=t2, in_=src2)

                # exp first: the se1/se2 -> c -> f -> g -> |g| chain is the
                # long dependency tail; squares/prod have slack
                nc.scalar.activation(out=megaE, in_=megaR[:, 0:2], func=Act.Exp)
                tree(fold_pool, megaE, 2, "e", statsA[:, 5:7, cols])

                # c = se1/se2 per row of this supertile
                rc = small_pool.tile([128, NB], F32, tag="rc", name="rc")
                nc.vector.reciprocal(out=rc, in_=statsA[:, 6, cols])
                cst = small_pool.tile([128, NB], F32, tag="cst", name="cst")
                _tt(nc, cst, statsA[:, 5, cols], rc, Alu.mult)
                # (c, c) bf16 pairs so the f-multiply's broadcast operand has
                # an innermost step-1 packed pair -> DVE 2x mode
                cpair = small_pool.tile([128, NB, 2], BF16, tag="cpair", name="cpair")
                nc.vector.tensor_copy(
                    out=cpair,
                    in_=cst.rearrange("p (b o) -> p b o", o=1).broadcast_to(
                        [128, NB, 2]
                    ),
                )

                # f = c*e2 in one tensor_tensor (in1 = (c,c) pairs repeated)
                f = fg_pool.tile([128, NB, N], BF16, tag="f", name="f")
                cb = cpair.rearrange("p b (o two) -> p b o two", o=1).broadcast_to(
                    [128, NB, N // 2, 2]
                )
                e2v = megaE[:, 1].rearrange("p b (h two) -> p b h two", two=2)
                _tt(nc, f.rearrange("p b (h two) -> p b h two", two=2), e2v, cb, Alu.mult)
                g = fg_pool.tile([128, NB, N], BF16, tag="g", name="g")
                _tt(nc, g, megaE[:, 0], f, Alu.subtract)
                ag = fg_pool.tile([128, 1, NB, N], BF16, tag="ag", name="ag")
                nc.scalar.activation(out=ag[:, 0], in_=g, func=Act.Abs)
                tree(fold_pool, ag, 1, "b", ta[:, :, cols])

                # slack work: squares, prod, and their tree
                nc.scalar.activation(
                    out=megaR[:, 2:4], in_=megaR[:, 0:2], func=Act.Square
                )
                _tt(nc, megaR[:, 4], t1, t2, Alu.mult)
                tree(fold_pool, megaR, 5, "r", statsA[:, 0:5, cols])


            if loop_k is not None:
                with tc.For_i(0, loop_k):
                    for st in range(NST):
                        one_supertile(st)
            elif unroll is not None:
                # python-unrolled repetitions (SWDGE DMA is not supported
                # inside For_i by this walrus build: InstIncSwdgeSem)
                for _rep in range(unroll):
                    for st in range(NST):
                        one_supertile(st)
            else:
                for st in range(NST):
                    one_supertile(st)

            # ---- per-row epilogue on [128, NCOLS] stat tiles ----
            def ep(name):
                return epi_pool.tile([128, NCOLS], F32, tag=name, name=name)

            s1a, s2a = statsA[:, 0, :], statsA[:, 1, :]
            q1a, q2a = statsA[:, 2, :], statsA[:, 3, :]
            s12a = statsA[:, 4, :]
            se1a = statsA[:, 5, :]

            # M2 = q - s^2/n ; num = s12 - s1*s2/n + n*EPS^2
            u1, m2_1 = ep("u1"), ep("m2_1")
            _tt(nc, u1, s1a, s1a, Alu.mult)
            nc.vector.scalar_tensor_tensor(
                out=m2_1, in0=u1, scalar=-1.0 / N, in1=q1a, op0=Alu.mult, op1=Alu.add
            )
            u2, m2_2 = ep("u1"), ep("m2_2")
            _tt(nc, u2, s2a, s2a, Alu.mult)
            nc.vector.scalar_tensor_tensor(
                out=m2_2, in0=u2, scalar=-1.0 / N, in1=q2a, op0=Alu.mult, op1=Alu.add
            )
            u, num, w = ep("u1"), ep("num"), ep("w")
            _tt(nc, u, s1a, s2a, Alu.mult)
            nc.vector.scalar_tensor_tensor(
                out=num, in0=u, scalar=-1.0 / N, in1=s12a, op0=Alu.mult, op1=Alu.add
            )
            _tt(nc, w, m2_1, m2_2, Alu.mult)

            # cor = (num + n*EPS^2) / (sqrt(w) + (n-1)*EPS), one Newton step
            # on the low-precision ACT sqrt
            sp, rsp, spn = ep("sp"), ep("rsp"), ep("u1")
            nc.scalar.activation(out=sp, in_=w, func=Act.Sqrt)
            nc.vector.reciprocal(out=rsp, in_=sp)
            _tt(nc, rsp, w, rsp, Alu.mult)
            _tt(nc, spn, sp, rsp, Alu.add)
            den, rden, cor = ep("den"), ep("rden"), ep("cor")
            nc.vector.tensor_scalar(
                out=den,
                in0=spn,
                scalar1=0.5,
                scalar2=(N - 1) * EPS,
                op0=Alu.mult,
                op1=Alu.add,
            )
            nc.vector.reciprocal(out=rden, in_=den)
            nc.vector.scalar_tensor_tensor(
                out=cor,
                in0=num,
                scalar=float(N) * EPS * EPS,
                in1=rden,
                op0=Alu.add,
                op1=Alu.mult,
            )
            c2, cor3 = ep("u1"), ep("cor3")
            _tt(nc, c2, cor, cor, Alu.mult)
            _tt(nc, cor3, c2, cor, Alu.mult)

            aa, lg, tl1 = ep("aa"), ep("lg"), ep("tl1")
            ln_bias = epi_pool.tile([128, 1], F32, tag="ln_bias", name="ln_bias")
            nc.vector.memset(ln_bias, 1.0 + EPS)
            nc.vector.scalar_tensor_tensor(
                out=aa, in0=cor3, scalar=-1.0, in1=cor3, op0=Alu.mult, op1=Alu.max
            )
            nc.scalar.activation(out=lg, in_=cor3, func=Act.Ln, bias=ln_bias)
            nc.vector.tensor_scalar(
                out=tl1,
                in0=lg,
                scalar1=-1.0,
                scalar2=float(np.log(2.0)),
                op0=Alu.mult,
                op1=Alu.add,
            )
            r1, tl2 = ep("r1"), ep("tl2")
            nc.vector.reciprocal(out=r1, in_=se1a)
            nc.vector.scalar_tensor_tensor(
                out=tl2, in0=ta[:, 0, :], scalar=1.0 / N, in1=r1, op0=Alu.mult, op1=Alu.mult
            )
            dd, pp, loss = ep("u1"), ep("pp"), ep("loss")
            _tt(nc, dd, tl1, tl2, Alu.subtract)
            _tt(nc, pp, aa, dd, Alu.mult)
            _tt(nc, loss, tl2, pp, Alu.add)

            part = epi_pool.tile([128, 1], F32, tag="part", name="part")
            nc.vector.reduce_sum(out=part, in_=loss, axis=mybir.AxisListType.X)
            nc.sync.dma_start(out=y[:, :], in_=part)

    split_waits(nc)
    return nc


_NC = None
_RUNNER = None


def _get_nc():
    global _NC
    if _NC is None:
        _NC = _build_program()
    return _NC


def _get_runner():
    """Compile the 8-core pjrt executable once and reuse across calls."""
    global _RUNNER
    if _RUNNER is not None:
        return _RUNNER
    import jax
    from jax.sharding import Mesh, PartitionSpec
    from jax.experimental.shard_map import shard_map
    from concourse.bass2jax import (
        _bass_exec_p,
        install_neuronx_cc_hook,
        partition_id_tensor,
    )

    install_neuronx_cc_hook()
    nc = _get_nc()
    partition_name = nc.partition_id_tensor.name if nc.partition_id_tensor else None
    in_names, out_names, out_avals, zero_outs = [], [], [], []
    for alloc in nc.m.functions[0].allocations:
        if not isinstance(alloc, mybir.MemoryLocationSet):
            continue
        name = alloc.memorylocations[0].name
        if alloc.kind == "ExternalInput":
            if name != partition_name:
                in_names.append(name)
        elif alloc.kind == "ExternalOutput":
            out_names.append(name)
            shape = tuple(alloc.tensor_shape)
            dtype = mybir.dt.np(alloc.dtype)
            out_avals.append(jax.core.ShapedArray(shape, dtype))
            zero_outs.append(np.zeros(shape, dtype))
    n_params = len(in_names)
    all_in_names = list(in_names) + out_names
    if partition_name is not None:
        all_in_names.append(partition_name)

    def _body(*args):
        operands = list(args)
        if partition_name is not None:
            operands.append(partition_id_tensor())
        outs = _bass_exec_p.bind(
            *operands,
            out_avals=tuple(out_avals),
            in_names=tuple(all_in_names),
            out_names=tuple(out_names),
            lowering_input_output_aliases=(),
            sim_require_finite=True,
            sim_require_nnan=True,
            nc=nc,
        )
        return tuple(outs)

    devices = jax.devices()[:N_CORES]
    mesh = Mesh(np.asarray(devices), ("core",))
    n_outs = len(out_names)
    in_specs = (PartitionSpec("core"),) * (n_params + n_outs)
    out_specs = (PartitionSpec("core"),) * n_outs
    sharded = jax.jit(
        shard_map(
            _body, mesh=mesh, in_specs=in_specs, out_specs=out_specs,
            check_rep=False,
        ),
        keep_unused=True,
    )
    zero_cat = [
        np.zeros((N_CORES * z.shape[0], *z.shape[1:]), z.dtype) for z in zero_outs
    ]

    def run(d1, d2):
        ins = {"d1": d1, "d2": d2}
        out = sharded(*(ins[nm] for nm in in_names), *zero_cat)
        y = np.asarray(out[out_names.index("y")])
        return y

    _RUNNER = run
    return _RUNNER


def kernel(distribution1, distribution2):
    d1 = np.ascontiguousarray(np.asarray(distribution1, dtype=np.float32))
    d2 = np.ascontiguousarray(np.asarray(distribution2, dtype=np.float32))
    assert d1.shape == (B, N) and d2.shape == (B, N)
    y = _get_runner()(d1, d2)  # [N_CORES*128, 1] partial sums
    return np.asarray([np.sum(y.astype(np.float64))], dtype=np.float32)
